# revision 22
# baseline (speedup 1.0000x reference)
"""EnsembleObsHead Trainium2 kernel.

Computes, for each of M=8 ensemble members m (each on its own NeuronCore):
    h   = x_m @ W1_m + b1_m          # [4096, 512] @ [512, 1024]
    h   = LayerNorm(h) * ln_w + ln_b
    h   = SiLU(h)
    out = h @ W2_m + b2_m            # [4096, 1024] @ [1024, 4096]

Sharding: ensemble dim M across the 8 cores (member-parallel; each core
also owns its member's batch slice). Matmuls run in float32r (full PE
rate, ~2^-12 relative rounding), everything else in fp32.

Layout per core:
  - x loaded naturally, transposed 128x128-wise on the PE -> xT (fp32r)
  - mm1: out[b, h] accumulated over 4 e-tiles; lhsT = xT, rhs = W1
  - LN in natural layout: per-partition stats via accum_out + Square
  - hT via PE transpose after SiLU; mm2: lhsT = hT, rhs = W2 (resident)
  - psum evac fused with b2 broadcast add (scalar_tensor_tensor)
"""
import sys

sys.path.insert(0, "/opt/trn_rl_repo")

from contextlib import ExitStack

import numpy as np

import concourse.bass as bass
import concourse.bacc as bacc
import concourse.tile as tile
from concourse import mybir
from concourse.bass_utils import run_bass_kernel_spmd
from concourse.masks import make_identity

M, E, H, V = 8, 512, 1024, 4096
BI = 4096
LN_EPS = 1e-5
N_CORES = 8

NB = BI // 128   # 32 b-tiles
NE = E // 128    # 4 e-tiles
NHC = H // 512   # 2 h-chunks
NK = H // 128    # 8 k-tiles
NV = V // 512    # 8 v-chunks

F32 = mybir.dt.float32
F32R = mybir.dt.float32r
ALU = mybir.AluOpType
ACTF = mybir.ActivationFunctionType

_CACHED_NC = None


def build():
    nc = bacc.Bacc("TRN2", target_bir_lowering=False, debug=False)

    # Matmul operands straight from DRAM are declared float32r (bits are
    # identical to fp32; walrus accepts DMA-ingested fp32r operands).
    x_d = nc.declare_dram_parameter("x", [BI, E], F32R, isOutput=False)
    w1_d = nc.declare_dram_parameter("w1", [E, H], F32R, isOutput=False)
    b1_d = nc.declare_dram_parameter("b1", [H], F32, isOutput=False)
    lnw_d = nc.declare_dram_parameter("lnw", [H], F32, isOutput=False)
    lnb_d = nc.declare_dram_parameter("lnb", [H], F32, isOutput=False)
    w2_d = nc.declare_dram_parameter("w2", [H, V], F32R, isOutput=False)
    b2_d = nc.declare_dram_parameter("b2", [V], F32, isOutput=False)
    out_d = nc.declare_dram_parameter("out", [BI, V], F32, isOutput=True)

    def bcast_row(dram_t, n):
        """AP reading a [n] DRAM row broadcast across 128 partitions."""
        a = dram_t.ap()
        return bass.AP(tensor=a.tensor, offset=a.offset, ap=[[0, 128], [1, n]])

    with tile.TileContext(nc) as tc, ExitStack() as ctx:
        consts = ctx.enter_context(tc.tile_pool(name="consts", bufs=1))
        xp = ctx.enter_context(tc.tile_pool(name="xp", bufs=2))
        xtp = ctx.enter_context(tc.tile_pool(name="xtp", bufs=2))
        hp = ctx.enter_context(tc.tile_pool(name="hp", bufs=2))
        up = ctx.enter_context(tc.tile_pool(name="up", bufs=1))
        htp = ctx.enter_context(tc.tile_pool(name="htp", bufs=2))
        outp = ctx.enter_context(tc.tile_pool(name="outp", bufs=3))
        statp = ctx.enter_context(tc.tile_pool(name="statp", bufs=3))
        ps1 = ctx.enter_context(
            tc.tile_pool(name="ps1", bufs=2, space=bass.MemorySpace.PSUM)
        )
        pst = ctx.enter_context(
            tc.tile_pool(name="pst", bufs=3, space=bass.MemorySpace.PSUM)
        )
        ps2 = ctx.enter_context(
            tc.tile_pool(name="ps2", bufs=3, space=bass.MemorySpace.PSUM)
        )

        # ---- resident constants ----
        identf = consts.tile([128, 128], F32)
        make_identity(nc, identf)
        ident = consts.tile([128, 128], F32R)
        nc.vector.tensor_copy(ident[:], identf[:])

        eps_t = consts.tile([128, 1], F32)
        nc.vector.memset(eps_t, LN_EPS)

        # x prefetch for the first tiles goes ahead of the weight loads in
        # the DMA queue so the PE can start transposing immediately.
        x_tiles = {}

        def load_x(b):
            # scalar HWDGE ring: keeps the sync ring clear for the W2 stream
            t = xp.tile([128, E], F32R, tag="x")
            nc.scalar.dma_start(t[:], x_d.ap()[b * 128 : (b + 1) * 128, :])
            x_tiles[b] = t

        load_x(0)

        # HAM warmup: ~4us of dummy matmuls on the identity while the DMAs
        # fill; the real matmuls then start at full clock.
        warm = ps2.tile([128, 512], F32, tag="p2")
        for _ in range(40):
            nc.tensor.matmul(
                warm[:, :128], ident[:], ident[:], start=True, stop=True
            )

        w1_t = []  # 4 tiles [128, 1024] fp32r
        for j in range(NE):
            t = consts.tile([128, H], F32R, tag=f"w1_{j}")
            nc.sync.dma_start(t[:], w1_d.ap()[j * 128 : (j + 1) * 128, :])
            w1_t.append(t)

        load_x(1)

        # W2 resident as 64 [128, 512] pieces, loaded v-chunk-major so the
        # first mm2 only waits on 2 MB, not the full 16 MB.
        w2_t = [[None] * NV for _ in range(NK)]
        for v in range(NV):
            for k in range(NK):
                t = consts.tile([128, 512], F32R, tag=f"w2_{k}_{v}")
                nc.sync.dma_start(
                    t[:],
                    w2_d.ap()[k * 128 : (k + 1) * 128, v * 512 : (v + 1) * 512],
                )
                w2_t[k][v] = t

        # broadcast rows ([128, n] tiles, same row on every partition)
        b1_bc = consts.tile([128, H], F32)
        nc.gpsimd.dma_start(b1_bc[:], bcast_row(b1_d, H))
        lnw_bc = consts.tile([128, H], F32)
        nc.gpsimd.dma_start(lnw_bc[:], bcast_row(lnw_d, H))
        lnb_bc = consts.tile([128, H], F32)
        nc.gpsimd.dma_start(lnb_bc[:], bcast_row(lnb_d, H))
        b2_bc = consts.tile([128, V], F32)
        nc.gpsimd.dma_start(b2_bc[:], bcast_row(b2_d, V))

        def emit_mm2(b, hT):
            # ---- mm2: out[b, :] = hT.T @ W2 + b2 ----
            for v in range(NV):
                p2 = ps2.tile([128, 512], F32, tag="p2")
                for k in range(NK):
                    nc.tensor.matmul(
                        p2[:],
                        hT[k // 4][:, (k % 4) * 128 : (k % 4 + 1) * 128],
                        w2_t[k][v][:],
                        start=(k == 0),
                        stop=(k == NK - 1),
                    )
                o = outp.tile([128, 512], F32, tag="o")
                nc.vector.scalar_tensor_tensor(
                    out=o[:], in0=p2[:], scalar=0.0,
                    in1=b2_bc[:, v * 512 : (v + 1) * 512],
                    op0=ALU.bypass, op1=ALU.add,
                )
                nc.scalar.dma_start(
                    out_d.ap()[b * 128 : (b + 1) * 128, v * 512 : (v + 1) * 512],
                    o[:],
                )

        def emit_front(b):
            """PE front half of tile b: x transpose, mm1, psum evac + stats
            accumulation. Emitted one tile AHEAD of the LN finale so the
            DVE/ACT streams pipeline across tiles."""
            x_t = x_tiles.pop(b)
            if b + 2 < NB:
                load_x(b + 2)

            pxt = pst.tile([128, E], F32R, tag="pt")
            for j in range(NE):
                nc.tensor.transpose(
                    pxt[:, j * 128 : (j + 1) * 128],
                    x_t[:, j * 128 : (j + 1) * 128],
                    ident[:],
                )
            xT = xtp.tile([128, E], F32R, tag="xT")
            nc.vector.tensor_copy(xT[:], pxt[:])

            # mm1: h[b, :] = xT.T @ W1; j outer so the stationary xT tile
            # is reused across both h-chunks
            hsb = hp.tile([128, H], F32, tag="hsb")
            acc = statp.tile([128, 2], F32, tag="acc")
            ssq = statp.tile([128, 2], F32, tag="ssq")
            p1s = [ps1.tile([128, 512], F32, tag="p1", name=f"p1_{hc}") for hc in range(NHC)]
            for j in range(NE):
                for hc in range(NHC):
                    nc.tensor.matmul(
                        p1s[hc][:],
                        xT[:, j * 128 : (j + 1) * 128],
                        w1_t[j][:, hc * 512 : (hc + 1) * 512],
                        start=(j == 0),
                        stop=(j == NE - 1),
                    )
            for hc in range(NHC):
                # evac + b1 add, accumulate row-sum
                nc.vector.scalar_tensor_tensor(
                    out=hsb[:, hc * 512 : (hc + 1) * 512],
                    in0=p1s[hc][:],
                    scalar=0.0,
                    in1=b1_bc[:, hc * 512 : (hc + 1) * 512],
                    op0=ALU.bypass,
                    op1=ALU.add,
                    accum_out=acc[:, hc : hc + 1],
                )
                # sum of squares for this chunk (ACT), scratch into psum
                nc.scalar.activation(
                    p1s[hc][:],
                    hsb[:, hc * 512 : (hc + 1) * 512],
                    ACTF.Square,
                    accum_out=ssq[:, hc : hc + 1],
                )
            return hsb, acc, ssq

        pending = None  # (b, hT) whose mm2 is emitted during next tile's LN
        fronts = {0: emit_front(0)}
        for b in range(NB):
            if b + 1 < NB:
                fronts[b + 1] = emit_front(b + 1)
            hsb, acc, ssq = fronts.pop(b)

            # ---- LN stats (per-partition tiny ops) ----
            st = statp.tile([128, 4], F32, tag="st")
            negmu = st[:, 0:1]
            mu2 = st[:, 1:2]
            var = st[:, 2:3]
            rsq = st[:, 3:4]
            nc.vector.tensor_reduce(
                negmu, acc[:], axis=mybir.AxisListType.X, op=ALU.add
            )
            nc.vector.tensor_scalar(negmu, negmu, -1.0 / H, None, ALU.mult)
            nc.vector.tensor_mul(mu2, negmu, negmu)
            # var = sumsq/H - mu^2
            sstot = statp.tile([128, 1], F32, tag="sstot")
            nc.vector.tensor_reduce(sstot, ssq[:], axis=mybir.AxisListType.X, op=ALU.add)
            nc.vector.scalar_tensor_tensor(
                out=var,
                in0=sstot[:],
                scalar=1.0 / H,
                in1=mu2,
                op0=ALU.mult,
                op1=ALU.subtract,
            )
            # rsq = 1/sqrt(var + eps)
            nc.scalar.activation(var, var, ACTF.Sqrt, bias=eps_t[:])
            nc.vector.reciprocal(rsq, var)

            # ---- normalize + ln scale/bias + SiLU (in-place passes) ----
            # hsb = (hsb + negmu) * lnw_bc ; hsb = hsb * rsq + lnb_bc ; silu
            nc.vector.scalar_tensor_tensor(
                out=hsb[:], in0=hsb[:], scalar=negmu, in1=lnw_bc[:],
                op0=ALU.add, op1=ALU.mult,
            )
            nc.vector.scalar_tensor_tensor(
                out=hsb[:], in0=hsb[:], scalar=rsq, in1=lnb_bc[:],
                op0=ALU.mult, op1=ALU.add,
            )
            hfinr = up.tile([128, H], F32R, tag="u")
            nc.scalar.activation(hfinr[:], hsb[:], ACTF.Silu)

            # Previous tile's mm2 goes here: it fills the PE while this
            # tile's LN chain runs on DVE/ACT.
            if pending is not None:
                emit_mm2(*pending)

            # ---- transpose h on PE -> hT (fp32r), packed 4-per-bank ----
            hT = []
            for half in range(2):
                pt = pst.tile([128, 512], F32R, tag="pt")
                for j in range(4):
                    k = half * 4 + j
                    nc.tensor.transpose(
                        pt[:, j * 128 : (j + 1) * 128],
                        hfinr[:, k * 128 : (k + 1) * 128],
                        ident[:],
                    )
                ht = htp.tile([128, 512], F32R, tag=f"hT{half}")
                nc.scalar.copy(ht[:], pt[:])
                hT.append(ht)

            pending = (b, hT)

        emit_mm2(*pending)

    nc.compile()
    return nc


def _get_nc():
    global _CACHED_NC
    if _CACHED_NC is None:
        _CACHED_NC = build()
    return _CACHED_NC


def kernel(x, W1, b1, ln_w, ln_b, W2, b2, _trace=False, _trace_kwargs=None):
    nc = _get_nc()
    x = np.ascontiguousarray(x, dtype=np.float32)
    in_maps = []
    for m in range(M):
        in_maps.append(
            {
                "x": x[m * BI : (m + 1) * BI],
                "w1": np.ascontiguousarray(W1[m], dtype=np.float32),
                "b1": np.ascontiguousarray(b1[m], dtype=np.float32),
                "lnw": np.ascontiguousarray(ln_w[m], dtype=np.float32),
                "lnb": np.ascontiguousarray(ln_b[m], dtype=np.float32),
                "w2": np.ascontiguousarray(W2[m], dtype=np.float32),
                "b2": np.ascontiguousarray(b2[m], dtype=np.float32),
            }
        )
    try:
        res = run_bass_kernel_spmd(
            nc, in_maps, list(range(N_CORES)), trace=_trace, **(_trace_kwargs or {})
        )
    except Exception:
        # transient NRT device errors have been observed; one retry suffices
        res = run_bass_kernel_spmd(
            nc, in_maps, list(range(N_CORES)), trace=_trace, **(_trace_kwargs or {})
        )
    out = np.concatenate([res.results[m]["out"] for m in range(M)], axis=0)
    kernel.last_exec_time_ns = res.exec_time_ns
    return out


if __name__ == "__main__":
    rng = np.random.default_rng(0)
    inputs = {
        "x": rng.standard_normal((M * BI, E), dtype=np.float32),
        "W1": (rng.uniform(-1, 1, (M, E, H)) / np.sqrt(E)).astype(np.float32),
        "b1": (rng.uniform(-1, 1, (M, H)) / np.sqrt(E)).astype(np.float32),
        "ln_w": np.ones((M, H), np.float32),
        "ln_b": np.zeros((M, H), np.float32),
        "W2": (rng.uniform(-1, 1, (M, H, V)) / np.sqrt(H)).astype(np.float32),
        "b2": (rng.uniform(-1, 1, (M, V)) / np.sqrt(H)).astype(np.float32),
    }
    out = kernel(**inputs)
    print("kernel out", out.shape, out.dtype)


# revision 24
# speedup vs baseline: 1.0073x; 1.0073x over previous
"""EnsembleObsHead Trainium2 kernel.

Computes, for each of M=8 ensemble members m (each on its own NeuronCore):
    h   = x_m @ W1_m + b1_m          # [4096, 512] @ [512, 1024]
    h   = LayerNorm(h) * ln_w + ln_b
    h   = SiLU(h)
    out = h @ W2_m + b2_m            # [4096, 1024] @ [1024, 4096]

Sharding: ensemble dim M across the 8 cores (member-parallel; each core
also owns its member's batch slice). Matmuls run in float32r (full PE
rate, ~2^-12 relative rounding), everything else in fp32.

Layout per core:
  - x loaded naturally, transposed 128x128-wise on the PE -> xT (fp32r)
  - mm1: out[b, h] accumulated over 4 e-tiles; lhsT = xT, rhs = W1
  - LN in natural layout: per-partition stats via accum_out + Square
  - hT via PE transpose after SiLU; mm2: lhsT = hT, rhs = W2 (resident)
  - psum evac fused with b2 broadcast add (scalar_tensor_tensor)
"""
import sys

sys.path.insert(0, "/opt/trn_rl_repo")

from contextlib import ExitStack

import numpy as np

import concourse.bass as bass
import concourse.bacc as bacc
import concourse.tile as tile
from concourse import mybir
from concourse.bass_utils import run_bass_kernel_spmd
from concourse.masks import make_identity

M, E, H, V = 8, 512, 1024, 4096
BI = 4096
LN_EPS = 1e-5
N_CORES = 8

NB = BI // 128   # 32 b-tiles
NE = E // 128    # 4 e-tiles
NHC = H // 512   # 2 h-chunks
NK = H // 128    # 8 k-tiles
NV = V // 512    # 8 v-chunks

F32 = mybir.dt.float32
F32R = mybir.dt.float32r
ALU = mybir.AluOpType
ACTF = mybir.ActivationFunctionType

_CACHED_NC = None


def build():
    nc = bacc.Bacc("TRN2", target_bir_lowering=False, debug=False)

    # Matmul operands straight from DRAM are declared float32r (bits are
    # identical to fp32; walrus accepts DMA-ingested fp32r operands).
    x_d = nc.declare_dram_parameter("x", [BI, E], F32R, isOutput=False)
    w1_d = nc.declare_dram_parameter("w1", [E, H], F32R, isOutput=False)
    b1_d = nc.declare_dram_parameter("b1", [H], F32, isOutput=False)
    lnw_d = nc.declare_dram_parameter("lnw", [H], F32, isOutput=False)
    lnb_d = nc.declare_dram_parameter("lnb", [H], F32, isOutput=False)
    w2_d = nc.declare_dram_parameter("w2", [H, V], F32R, isOutput=False)
    b2_d = nc.declare_dram_parameter("b2", [V], F32, isOutput=False)
    out_d = nc.declare_dram_parameter("out", [BI, V], F32, isOutput=True)

    def bcast_row(dram_t, n):
        """AP reading a [n] DRAM row broadcast across 128 partitions."""
        a = dram_t.ap()
        return bass.AP(tensor=a.tensor, offset=a.offset, ap=[[0, 128], [1, n]])

    with tile.TileContext(nc) as tc, ExitStack() as ctx:
        consts = ctx.enter_context(tc.tile_pool(name="consts", bufs=1))
        xp = ctx.enter_context(tc.tile_pool(name="xp", bufs=2))
        xtp = ctx.enter_context(tc.tile_pool(name="xtp", bufs=2))
        hp = ctx.enter_context(tc.tile_pool(name="hp", bufs=2))
        up = ctx.enter_context(tc.tile_pool(name="up", bufs=1))
        htp = ctx.enter_context(tc.tile_pool(name="htp", bufs=2))
        outp = ctx.enter_context(tc.tile_pool(name="outp", bufs=3))
        statp = ctx.enter_context(tc.tile_pool(name="statp", bufs=3))
        ps1 = ctx.enter_context(
            tc.tile_pool(name="ps1", bufs=2, space=bass.MemorySpace.PSUM)
        )
        pst = ctx.enter_context(
            tc.tile_pool(name="pst", bufs=3, space=bass.MemorySpace.PSUM)
        )
        ps2 = ctx.enter_context(
            tc.tile_pool(name="ps2", bufs=3, space=bass.MemorySpace.PSUM)
        )

        # ---- resident constants ----
        identf = consts.tile([128, 128], F32)
        make_identity(nc, identf)
        ident = consts.tile([128, 128], F32R)
        nc.vector.tensor_copy(ident[:], identf[:])

        eps_t = consts.tile([128, 1], F32)
        nc.vector.memset(eps_t, LN_EPS)

        # x prefetch for the first tiles goes ahead of the weight loads in
        # the DMA queue so the PE can start transposing immediately.
        x_tiles = {}

        def load_x(b):
            # scalar HWDGE ring: keeps the sync ring clear for the W2 stream
            t = xp.tile([128, E], F32R, tag="x")
            nc.scalar.dma_start(t[:], x_d.ap()[b * 128 : (b + 1) * 128, :])
            x_tiles[b] = t

        load_x(0)

        # HAM warmup: ~4us of dummy matmuls on the identity while the DMAs
        # fill; the real matmuls then start at full clock.
        warm = ps2.tile([128, 512], F32, tag="p2")
        for _ in range(40):
            nc.tensor.matmul(
                warm[:, :128], ident[:], ident[:], start=True, stop=True
            )

        w1_t = []  # 4 tiles [128, 1024] fp32r
        for j in range(NE):
            t = consts.tile([128, H], F32R, tag=f"w1_{j}")
            nc.sync.dma_start(t[:], w1_d.ap()[j * 128 : (j + 1) * 128, :])
            w1_t.append(t)

        load_x(1)

        # W2 resident as 64 [128, 512] pieces, loaded v-chunk-major so the
        # first mm2 only waits on 2 MB, not the full 16 MB.
        w2_t = [[None] * NV for _ in range(NK)]
        for v in range(NV):
            for k in range(NK):
                t = consts.tile([128, 512], F32R, tag=f"w2_{k}_{v}")
                nc.sync.dma_start(
                    t[:],
                    w2_d.ap()[k * 128 : (k + 1) * 128, v * 512 : (v + 1) * 512],
                )
                w2_t[k][v] = t

        # broadcast rows ([128, n] tiles, same row on every partition)
        b1_bc = consts.tile([128, H], F32)
        nc.gpsimd.dma_start(b1_bc[:], bcast_row(b1_d, H))
        lnw_bc = consts.tile([128, H], F32)
        nc.gpsimd.dma_start(lnw_bc[:], bcast_row(lnw_d, H))
        lnb_bc = consts.tile([128, H], F32)
        nc.gpsimd.dma_start(lnb_bc[:], bcast_row(lnb_d, H))
        b2_bc = consts.tile([128, V], F32)
        nc.gpsimd.dma_start(b2_bc[:], bcast_row(b2_d, V))

        def emit_mm2(b, hT):
            # ---- mm2: out[b, :] = hT.T @ W2 + b2 ----
            for v in range(NV):
                p2 = ps2.tile([128, 512], F32, tag="p2")
                for k in range(NK):
                    nc.tensor.matmul(
                        p2[:],
                        hT[k // 4][:, (k % 4) * 128 : (k % 4 + 1) * 128],
                        w2_t[k][v][:],
                        start=(k == 0),
                        stop=(k == NK - 1),
                    )
                o = outp.tile([128, 512], F32, tag="o")
                nc.vector.scalar_tensor_tensor(
                    out=o[:], in0=p2[:], scalar=0.0,
                    in1=b2_bc[:, v * 512 : (v + 1) * 512],
                    op0=ALU.bypass, op1=ALU.add,
                )
                nc.scalar.dma_start(
                    out_d.ap()[b * 128 : (b + 1) * 128, v * 512 : (v + 1) * 512],
                    o[:],
                )

        def emit_front(b):
            """PE front half of tile b: x transpose, mm1, psum evac + stats
            accumulation. Emitted one tile AHEAD of the LN finale so the
            DVE/ACT streams pipeline across tiles."""
            x_t = x_tiles.pop(b)
            if b + 2 < NB:
                load_x(b + 2)

            pxt = pst.tile([128, E], F32R, tag="pt")
            for j in range(NE):
                nc.tensor.transpose(
                    pxt[:, j * 128 : (j + 1) * 128],
                    x_t[:, j * 128 : (j + 1) * 128],
                    ident[:],
                )
            xT = xtp.tile([128, E], F32R, tag="xT")
            nc.vector.tensor_copy(xT[:], pxt[:])

            # mm1: h[b, :] = xT.T @ W1; j outer so the stationary xT tile
            # is reused across both h-chunks
            hsb = hp.tile([128, H], F32, tag="hsb")
            acc = statp.tile([128, 2], F32, tag="acc")
            ssq = statp.tile([128, 2], F32, tag="ssq")
            p1s = [ps1.tile([128, 512], F32, tag="p1", name=f"p1_{hc}") for hc in range(NHC)]
            for j in range(NE):
                for hc in range(NHC):
                    nc.tensor.matmul(
                        p1s[hc][:],
                        xT[:, j * 128 : (j + 1) * 128],
                        w1_t[j][:, hc * 512 : (hc + 1) * 512],
                        start=(j == 0),
                        stop=(j == NE - 1),
                    )
            for hc in range(NHC):
                # evac + b1 add, accumulate row-sum
                nc.vector.scalar_tensor_tensor(
                    out=hsb[:, hc * 512 : (hc + 1) * 512],
                    in0=p1s[hc][:],
                    scalar=0.0,
                    in1=b1_bc[:, hc * 512 : (hc + 1) * 512],
                    op0=ALU.bypass,
                    op1=ALU.add,
                    accum_out=acc[:, hc : hc + 1],
                )
                # sum of squares for this chunk (ACT), scratch into psum
                nc.scalar.activation(
                    p1s[hc][:],
                    hsb[:, hc * 512 : (hc + 1) * 512],
                    ACTF.Square,
                    accum_out=ssq[:, hc : hc + 1],
                )
            return hsb, acc, ssq

        pending = None  # (b, hT) whose mm2 is emitted during next tile's LN
        fronts = {0: emit_front(0)}
        for b in range(NB):
            if b + 1 < NB:
                fronts[b + 1] = emit_front(b + 1)
            hsb, acc, ssq = fronts.pop(b)

            # ---- LN stats (per-partition tiny ops) ----
            st = statp.tile([128, 4], F32, tag="st")
            negmu = st[:, 0:1]
            mu2 = st[:, 1:2]
            var = st[:, 2:3]
            rsq = st[:, 3:4]
            nc.vector.tensor_reduce(
                negmu, acc[:], axis=mybir.AxisListType.X, op=ALU.add
            )
            nc.vector.tensor_scalar(negmu, negmu, -1.0 / H, None, ALU.mult)
            nc.vector.tensor_mul(mu2, negmu, negmu)
            # var = sumsq/H - mu^2
            sstot = statp.tile([128, 1], F32, tag="sstot")
            nc.vector.tensor_reduce(sstot, ssq[:], axis=mybir.AxisListType.X, op=ALU.add)
            nc.vector.scalar_tensor_tensor(
                out=var,
                in0=sstot[:],
                scalar=1.0 / H,
                in1=mu2,
                op0=ALU.mult,
                op1=ALU.subtract,
            )
            # rsq = 1/sqrt(var + eps)
            nc.scalar.activation(var, var, ACTF.Sqrt, bias=eps_t[:])
            nc.vector.reciprocal(rsq, var)

            # ---- normalize + ln scale/bias + SiLU (in-place passes) ----
            # hsb = (hsb + negmu) * lnw_bc ; hsb = hsb * rsq + lnb_bc ; silu
            nc.vector.scalar_tensor_tensor(
                out=hsb[:], in0=hsb[:], scalar=negmu, in1=lnw_bc[:],
                op0=ALU.add, op1=ALU.mult,
            )
            nc.vector.scalar_tensor_tensor(
                out=hsb[:], in0=hsb[:], scalar=rsq, in1=lnb_bc[:],
                op0=ALU.mult, op1=ALU.add,
            )
            hfinr = up.tile([128, H], F32R, tag="u")
            nc.scalar.activation(hfinr[:], hsb[:], ACTF.Silu)

            # Previous tile's mm2 goes here: it fills the PE while this
            # tile's LN chain runs on DVE/ACT.
            if pending is not None:
                emit_mm2(*pending)

            # ---- transpose h on PE -> hT (fp32r), packed 4-per-bank ----
            hT = []
            for half in range(2):
                pt = pst.tile([128, 512], F32R, tag="pt")
                for j in range(4):
                    k = half * 4 + j
                    nc.tensor.transpose(
                        pt[:, j * 128 : (j + 1) * 128],
                        hfinr[:, k * 128 : (k + 1) * 128],
                        ident[:],
                    )
                ht = htp.tile([128, 512], F32R, tag=f"hT{half}")
                nc.scalar.copy(ht[:], pt[:])
                hT.append(ht)

            pending = (b, hT)

        emit_mm2(*pending)

    nc.compile()
    return nc


def _get_nc():
    global _CACHED_NC
    if _CACHED_NC is None:
        _CACHED_NC = build()
    return _CACHED_NC


def kernel(x, W1, b1, ln_w, ln_b, W2, b2, _trace=False, _trace_kwargs=None):
    nc = _get_nc()
    x = np.ascontiguousarray(x, dtype=np.float32)
    in_maps = []
    for m in range(M):
        in_maps.append(
            {
                "x": x[m * BI : (m + 1) * BI],
                "w1": np.ascontiguousarray(W1[m], dtype=np.float32),
                "b1": np.ascontiguousarray(b1[m], dtype=np.float32),
                "lnw": np.ascontiguousarray(ln_w[m], dtype=np.float32),
                "lnb": np.ascontiguousarray(ln_b[m], dtype=np.float32),
                "w2": np.ascontiguousarray(W2[m], dtype=np.float32),
                "b2": np.ascontiguousarray(b2[m], dtype=np.float32),
            }
        )
    try:
        res = run_bass_kernel_spmd(
            nc, in_maps, list(range(N_CORES)), trace=_trace, **(_trace_kwargs or {})
        )
    except Exception:
        # transient NRT device errors have been observed; one retry suffices
        res = run_bass_kernel_spmd(
            nc, in_maps, list(range(N_CORES)), trace=_trace, **(_trace_kwargs or {})
        )
    out = np.concatenate([res.results[m]["out"] for m in range(M)], axis=0)
    kernel.last_exec_time_ns = res.exec_time_ns
    return out


if __name__ == "__main__":
    rng = np.random.default_rng(0)
    inputs = {
        "x": rng.standard_normal((M * BI, E), dtype=np.float32),
        "W1": (rng.uniform(-1, 1, (M, E, H)) / np.sqrt(E)).astype(np.float32),
        "b1": (rng.uniform(-1, 1, (M, H)) / np.sqrt(E)).astype(np.float32),
        "ln_w": np.ones((M, H), np.float32),
        "ln_b": np.zeros((M, H), np.float32),
        "W2": (rng.uniform(-1, 1, (M, H, V)) / np.sqrt(H)).astype(np.float32),
        "b2": (rng.uniform(-1, 1, (M, V)) / np.sqrt(H)).astype(np.float32),
    }
    out = kernel(**inputs)
    print("kernel out", out.shape, out.dtype)
